# revision 7
# baseline (speedup 1.0000x reference)
"""Trainium2 Bass kernel for a 2-layer GRU (B=256, T=256, D=128, H=512) + FC head.

Strategy: data-parallel over batch (B=32 per core, 8 cores), single SPMD launch.
Everything stays on-chip after the initial weight/x loads.

v5 changes over the original baseline:
  - r,z-gate matmuls use fp8e4 stationary weights (bf16 moving) -> ~2x faster
    LDWEIGHTS on the LDW-bound FD=32 recurrent matmuls. (GRU_RZ8=0 disables.)
  - bias adds (hnp/xnp/hnpb) + the xg1 chunk copy moved to the idle GpSimd
    (Pool) engine; DVE keeps only the critical-path ops.
  - h-update uses 3 ops: h' = n + z*(h - n)  (was 4).
  - gate PSUM tiles packed [128,16,32] (1 bank) and double-buffered so PE
    doesn't serialize on the DVE/ACT chain every step.
  - L1 lag increased to 6 steps (xg1 chunk copy latency lives on GpSimd).

Layouts (per core, local batch 32):
  - All recurrent tensors live "transposed": gh.T = [gate_dim on partitions,
    batch free]. m-tiles: r = 0-3, z = 4-7, n = 8-11.
    h.T stored as [128, k, 32]: h-slab k (h dims 128k..128k+127) at [:, k, :].
  - x passed from host pre-transposed: xT [128(D), T, 32(batch)].
  - Weights pre-transposed on host (W.T K-tiles); rz m-tiles also in fp8.
  - L0 input projection folded into each step's PSUM accumulation group.
  - L1 input projection computed in CH-step chunks from the y0 ring into
    PSUM, then copied (+bias) to an SBUF ring by GpSimd.
"""

import os
import sys

sys.path.insert(0, "/opt/trn_rl_repo")

import ml_dtypes
import numpy as np

import concourse.bass as bass
import concourse.tile as tile
from concourse import mybir
from concourse.bass_utils import run_bass_kernel_spmd

AF = mybir.ActivationFunctionType
ALU = mybir.AluOpType
F32 = mybir.dt.float32
FP8 = mybir.dt.float8e4

B, T, D, H, C = 256, 256, 128, 512, 10
NCORES = 8
BL = B // NCORES          # 32 batch per core
G3 = 3 * H                # 1536
NK = H // 128             # 4 h k-tiles
NM = G3 // 128            # 12 gate m-tiles
RING = 16                 # y0 ring slots (steps)
CH = 4                    # xg1 chunk size (steps)
RB = 3                    # xg1 ring chunk slots
LAG = 6                   # L1 step lag behind L0

_CACHE = {}


def _split_multiwaits(nc):
    """Walrus/HW allow a single sync-wait per engine instruction. Tile can
    emit several; hoist extras into same-engine NoOps placed just before."""
    import json as _json
    import types as _types

    d = _json.loads(mybir.module_to_json_bytes(nc.m))
    nsw = 0
    for fn in d["functions"]:
        for blk in fn["blocks"]:
            out = []
            for ins in blk["instructions"]:
                si = ins.get("sync_info") or {}
                ow = si.get("on_wait") or []
                if len(ow) > 1:
                    for w in ow[:-1]:
                        out.append({
                            "engine": ins["engine"],
                            "ins": [],
                            "outs": [],
                            "name": f"I-SW-{nsw}",
                            "opcode": "NoOp",
                            "sync_info": {"on_update": [], "on_wait": [w]},
                        })
                        nsw += 1
                    si["on_wait"] = [ow[-1]]
                out.append(ins)
            blk["instructions"] = out
    blob = _json.dumps(d).encode()
    nc.to_json_bytes = _types.MethodType(lambda self: blob, nc)
    return nsw


def _build(dt_w, n_steps, rz8):
    """Build the Bass program. dt_w: weight/x/h/gate dtype. Returns nc."""
    DT = dt_w
    nc = bass.Bass("TRN2", target_bir_lowering=False, debug=False, num_devices=NCORES)

    nwf = 8 * BL + 3 * NK * BL + NM * CH * BL + 2
    fmul = 4 // mybir.dt.size(DT)          # DT cols per f32 value
    n8 = (8 * 128 + 2 * NK * 8 * 128) // 2 if rz8 else 0  # fp8 bytes / 2 per col
    nwd = n_steps * BL + G3 + 3 * NK * G3 + NK * 128 + C + 2 + nwf * fmul + n8
    d_wb = nc.dram_tensor("wb", [128, nwd], DT, kind="ExternalInput").ap()
    d_out = nc.dram_tensor("out", [C, BL], F32, kind="ExternalOutput").ap()

    with tile.TileContext(nc) as tc:
        with (
            tc.tile_pool(name="w", bufs=1) as wp,
            tc.tile_pool(name="ring", bufs=1) as ringp,
            tc.tile_pool(name="h1", bufs=3) as h1p,
            tc.tile_pool(name="g", bufs=3) as gp,
            tc.tile_pool(name="g2", bufs=3) as gp2,
            tc.tile_pool(name="p0", bufs=2, space="PSUM") as p0p,
            tc.tile_pool(name="p1", bufs=2, space="PSUM") as p1p,
            tc.tile_pool(name="pxg1", bufs=1, space="PSUM") as pxg1p,
        ):
            # ---- one persistent SBUF blob, ONE load DMA (HW allows only a
            # single sync-wait per instruction, so all consumers join on it) ----
            wb = wp.tile([128, nwd], DT)
            y0r = ringp.tile([128, RING, NK, BL], DT)   # y0 / h0 ring
            xg1r = ringp.tile([128, RB, NM, CH, BL], DT)  # xg1 chunk ring
            h1init = ringp.tile([128, NK, BL], DT)
            nc.sync.dma_start(wb[:], d_wb[:])

            # blob views
            o = 0
            def take(n):
                nonlocal o
                a, o = o, o + n
                return a
            o_xT = take(n_steps * BL)
            o_wih0 = take(G3)
            o_whh0 = take(NK * G3)
            o_wih1 = take(NK * G3)
            o_whh1 = take(NK * G3)
            o_fc1w = take(NK * 128)
            o_fc2w = take(C + 2)
            o_f32 = take(nwf * fmul)
            o_8 = take(n8)
            xT = wb[:, o_xT:o_xT + n_steps * BL].rearrange("p (t b) -> p t b", b=BL)
            fc2w = wb[:, o_fc2w:o_fc2w + C]
            fbt = wb[:, o_f32:o_f32 + nwf * fmul].bitcast(F32)
            if rz8:
                w8 = wb[:, o_8:o_8 + n8].bitcast(FP8)   # [128, 2*n8]
            o = 0
            o_brz0 = take(8 * BL)
            o_bxn0 = take(NK * BL)
            o_bhn0 = take(NK * BL)
            o_bhn1 = take(NK * BL)
            o_bx1 = take(NM * CH * BL)
            o_fc1b = take(1)
            o_fc2b = take(1)
            brz0 = fbt[:, o_brz0:o_brz0 + 8 * BL].rearrange("p (m b) -> p m b", b=BL)
            bxn0 = fbt[:, o_bxn0:o_bxn0 + NK * BL].rearrange("p (m b) -> p m b", b=BL)
            bhn0 = fbt[:, o_bhn0:o_bhn0 + NK * BL].rearrange("p (m b) -> p m b", b=BL)
            bhn1 = fbt[:, o_bhn1:o_bhn1 + NK * BL].rearrange("p (m b) -> p m b", b=BL)
            bx1 = fbt[:, o_bx1:o_bx1 + NM * CH * BL].rearrange(
                "p (m s b) -> p m s b", s=CH, b=BL)
            fc1b = fbt[:, o_fc1b:o_fc1b + 1]
            fc2b = fbt[:, o_fc2b:o_fc2b + 1]

            def wih0_t(m):
                return wb[:, o_wih0 + m * 128:o_wih0 + (m + 1) * 128]
            def whh0_t(k, m):
                return wb[:, o_whh0 + k * G3 + m * 128:o_whh0 + k * G3 + (m + 1) * 128]
            def wih1_t(k, m):
                return wb[:, o_wih1 + k * G3 + m * 128:o_wih1 + k * G3 + (m + 1) * 128]
            def whh1_t(k, m):
                return wb[:, o_whh1 + k * G3 + m * 128:o_whh1 + k * G3 + (m + 1) * 128]
            def fc1w_t(k):
                return wb[:, o_fc1w + k * 128:o_fc1w + (k + 1) * 128]
            # fp8 rz stationaries: wih0rz (8 m), whh0rz (4k x 8m), whh1rz
            if rz8:
                def wih0rz_t(m):
                    return w8[:, m * 128:(m + 1) * 128]
                def whh0rz_t(k, m):
                    b0 = 8 * 128 + (k * 8 + m) * 128
                    return w8[:, b0:b0 + 128]
                def whh1rz_t(k, m):
                    b0 = 8 * 128 + NK * 8 * 128 + (k * 8 + m) * 128
                    return w8[:, b0:b0 + 128]
            else:
                wih0rz_t = wih0_t
                whh0rz_t, whh1rz_t = whh0_t, whh1_t

            # HW allows only ONE sync-wait per instruction. Prime DVE/ACT/Pool
            # with a tiny read of the blob so their clocks observe the load-DMA
            # sem once; afterwards every instruction needs at most one wait.
            prdve = gp.tile([1, 4], DT, tag="prime")
            nc.vector.tensor_copy(prdve[:], wb[0:1, 0:4])
            pract = gp.tile([1, 4], DT, tag="prime2")
            nc.scalar.copy(pract[:], wb[0:1, 0:4])
            prgps = gp.tile([1, 4], DT, tag="prime3")
            nc.gpsimd.tensor_copy(prgps[:], wb[0:1, 0:4])
            nc.vector.memset(y0r[:, RING - 1], 0.0)  # h0(t=-1) = 0
            nc.vector.memset(h1init[:], 0.0)

            def l0_step(t):
                rs = (t + RING - 1) % RING   # h_old ring slot
                ws = t % RING                # h_new ring slot
                p0 = p0p.tile([128, 16, BL], F32, tag="p0")
                prz, phn, pxn = p0[:, 0:8], p0[:, 8:12], p0[:, 12:16]
                for m in range(8):           # r, z
                    o = prz[:, m]
                    nc.tensor.matmul(o, wih0rz_t(m),
                                     xT[:, t], start=True, stop=False)
                    for k in range(NK):
                        nc.tensor.matmul(
                            o, whh0rz_t(k, m),
                            y0r[:, rs, k], start=False, stop=(k == NK - 1))
                for m in range(8, NM):       # n: keep x-part and h-part separate
                    j = m - 8
                    nc.tensor.matmul(pxn[:, j], wih0_t(m),
                                     xT[:, t], start=True, stop=True)
                    for k in range(NK):
                        nc.tensor.matmul(
                            phn[:, j], whh0_t(k, m),
                            y0r[:, rs, k], start=(k == 0), stop=(k == NK - 1))
                t0 = gp.tile([128, 8, BL], DT, tag="t0")
                nc.vector.tensor_add(t0[:], prz[:], brz0[:])
                sig = gp.tile([128, 8, BL], DT, tag="sig")
                nc.scalar.activation(sig[:], t0[:], AF.Sigmoid)
                hnp = gp.tile([128, NK, BL], DT, tag="hnp")
                nc.vector.tensor_add(hnp[:], phn[:], bhn0[:])
                xnp = gp.tile([128, NK, BL], DT, tag="xnp")
                nc.vector.tensor_add(xnp[:], pxn[:], bxn0[:])
                t2 = gp.tile([128, NK, BL], DT, tag="t2")
                nc.gpsimd.tensor_mul(t2[:], sig[:, 0:4], hnp[:])
                t3 = gp.tile([128, NK, BL], DT, tag="t3")
                nc.gpsimd.tensor_add(t3[:], t2[:], xnp[:])
                ntl = gp.tile([128, NK, BL], DT, tag="ntl")
                nc.scalar.activation(ntl[:], t3[:], AF.Tanh)
                d = gp.tile([128, NK, BL], DT, tag="d")
                nc.vector.tensor_sub(d[:], y0r[:, rs], ntl[:])
                e = gp.tile([128, NK, BL], DT, tag="e")
                nc.vector.tensor_mul(e[:], sig[:, 4:8], d[:])
                nc.vector.tensor_add(y0r[:, ws], ntl[:], e[:])

            def xg1_chunk(c):
                """project y0 steps [CH*c, CH*c+CH) -> xg1 ring chunk (+ bias)."""
                r0 = (CH * c) % RING
                pxg1 = pxg1p.tile([128, NM, CH, BL], F32)
                for m in range(NM):
                    for k in range(NK):
                        nc.tensor.matmul(
                            pxg1[:, m], wih1_t(k, m),
                            y0r[:, r0:r0 + CH, k], start=(k == 0),
                            stop=(k == NK - 1))
                nc.vector.tensor_add(xg1r[:, c % RB], pxg1[:], bx1[:])

            def l1_step(tau, h_old):
                s = tau % CH
                rb = (tau // CH) % RB
                p1 = p1p.tile([128, 12, BL], F32, tag="p1")
                prz, phn = p1[:, 0:8], p1[:, 8:12]
                for m in range(8):
                    for k in range(NK):
                        nc.tensor.matmul(
                            prz[:, m], whh1rz_t(k, m),
                            h_old[:, k], start=(k == 0), stop=(k == NK - 1))
                for m in range(8, NM):
                    for k in range(NK):
                        nc.tensor.matmul(
                            phn[:, m - 8], whh1_t(k, m),
                            h_old[:, k], start=(k == 0), stop=(k == NK - 1))
                t0 = gp2.tile([128, 8, BL], DT, tag="t0b")
                nc.vector.tensor_add(t0[:], prz[:], xg1r[:, rb, 0:8, s])
                sig = gp2.tile([128, 8, BL], DT, tag="sigb")
                nc.scalar.activation(sig[:], t0[:], AF.Sigmoid)
                hnp = gp2.tile([128, NK, BL], DT, tag="hnpb")
                nc.vector.tensor_add(hnp[:], phn[:], bhn1[:])
                t2 = gp2.tile([128, NK, BL], DT, tag="t2b")
                nc.gpsimd.tensor_mul(t2[:], sig[:, 0:4], hnp[:])
                t3 = gp2.tile([128, NK, BL], DT, tag="t3b")
                nc.gpsimd.tensor_add(t3[:], t2[:], xg1r[:, rb, 8:12, s])
                ntl = gp2.tile([128, NK, BL], DT, tag="ntlb")
                nc.scalar.activation(ntl[:], t3[:], AF.Tanh)
                d = gp2.tile([128, NK, BL], DT, tag="db")
                nc.vector.tensor_sub(d[:], h_old[:], ntl[:])
                e = gp2.tile([128, NK, BL], DT, tag="eb")
                nc.vector.tensor_mul(e[:], sig[:, 4:8], d[:])
                h_new = h1p.tile([128, NK, BL], DT, tag="h1")
                nc.vector.tensor_add(h_new[:], ntl[:], e[:])
                return h_new

            # ---- main pipeline: L1 lags L0 by LAG steps ----
            h1 = h1init
            for t in range(n_steps):
                l0_step(t)
                if t % CH == CH - 1:
                    xg1_chunk(t // CH)
                tau = t - LAG
                if tau >= 0:
                    h1 = l1_step(tau, h1)
            for tau in range(n_steps - LAG, n_steps):
                h1 = l1_step(tau, h1)

            # ---- FC head ----
            hr = gp.tile([128, NK, BL], DT, tag="hr")
            nc.scalar.activation(hr[:], h1[:], AF.Relu)
            pfct = p0p.tile([128, 16, BL], F32, tag="p0")
            pfc = pfct[:, 0]
            for k in range(NK):
                nc.tensor.matmul(pfc, fc1w_t(k), hr[:, k],
                                 start=(k == 0), stop=(k == NK - 1))
            o1 = gp.tile([128, BL], DT, tag="o1")
            nc.scalar.activation(o1[:], pfc, AF.Relu, bias=fc1b[:])
            pfc2t = p1p.tile([128, 12, BL], F32, tag="p1")
            pfc2 = pfc2t[0:C, 0]
            nc.tensor.matmul(pfc2, fc2w[:], o1[:], start=True, stop=True)
            ofin = gp.tile([C, BL], F32, tag="ofin")
            nc.scalar.activation(ofin[:], pfc2, AF.Identity, bias=fc2b[0:C, :])
            nc.sync.dma_start(d_out[:], ofin[:])

    return nc


def _prep_inputs(inputs, dt_np, n_steps, rz8):
    """Host-side layout prep: pack per-core DT blob + shared F32 blob."""
    f32 = np.float32
    fp8 = ml_dtypes.float8_e4m3
    x = inputs["x"][:, :n_steps, :]
    b_ih0, b_hh0 = inputs["b_ih0"].astype(f32), inputs["b_hh0"].astype(f32)
    b_ih1, b_hh1 = inputs["b_ih1"].astype(f32), inputs["b_hh1"].astype(f32)

    def kt(W, dt):  # [3H, K] -> [128, NKw*3H] K-tiles of W.T side by side
        Wt = W.T.astype(dt)  # [K, 3H]
        nk = W.shape[1] // 128
        return Wt.reshape(nk, 128, G3).transpose(1, 0, 2).reshape(128, nk * G3)

    wih0 = inputs["W_ih0"].T.astype(dt_np)               # [128, 1536]
    whh0, wih1, whh1 = (kt(inputs["W_hh0"], dt_np), kt(inputs["W_ih1"], dt_np),
                        kt(inputs["W_hh1"], dt_np))
    fc1w = (inputs["fc1_w"].T.astype(dt_np)              # [512, 128] -> [128, 4*128]
            .reshape(NK, 128, 128).transpose(1, 0, 2).reshape(128, NK * 128))
    fc2w = inputs["fc2_w"].T.astype(dt_np)               # [128, 10]

    def bcast(bias, nm, reps):  # [nm*128] -> [128, nm*reps]
        return np.broadcast_to(
            bias.reshape(nm, 128).T[:, :, None], (128, nm, reps)
        ).reshape(128, nm * reps)

    brz0 = bcast((b_ih0 + b_hh0)[:1024], 8, BL)
    bxn0 = bcast(b_ih0[1024:], NK, BL)
    bhn0 = bcast(b_hh0[1024:], NK, BL)
    bhn1 = bcast(b_hh1[1024:], NK, BL)
    bfull = b_ih1.copy()
    bfull[:1024] += b_hh1[:1024]
    bx1 = bcast(bfull, NM, CH * BL)
    fc1b = inputs["fc1_b"].astype(f32).reshape(128, 1)
    fc2b = np.zeros((128, 1), f32)
    fc2b[:C, 0] = inputs["fc2_b"].astype(f32)

    fb = np.ascontiguousarray(np.concatenate(
        [brz0, bxn0, bhn0, bhn1, bx1, fc1b, fc2b], axis=1)).astype(f32)
    fb_dt = fb.view(np.uint8).reshape(128, -1).view(dt_np)  # raw bytes as DT cols
    pad = np.zeros((128, 2), dt_np)

    parts = [wih0, whh0, wih1, whh1, fc1w, fc2w, pad, fb_dt]
    if rz8:
        # fp8 rz stationaries, packed as raw bytes into DT columns
        wih0rz = inputs["W_ih0"].T.astype(fp8)[:, :1024]         # [128, 8*128]
        def ktrz(W):  # [128, NKw*3H] k-major rz tiles [k, m<8]
            Wt = W.T.astype(fp8)
            nk = W.shape[1] // 128
            return (Wt.reshape(nk, 128, G3)[:, :, :1024]
                    .transpose(1, 0, 2).reshape(128, nk * 1024))
        whh0rz = ktrz(inputs["W_hh0"])
        whh1rz = ktrz(inputs["W_hh1"])
        w8 = np.ascontiguousarray(
            np.concatenate([wih0rz, whh0rz, whh1rz], axis=1))
        parts.append(w8.view(np.uint8).reshape(128, -1).view(dt_np))

    wtail = np.concatenate(parts, axis=1)
    in_maps = []
    for c in range(NCORES):
        xc = x[c * BL:(c + 1) * BL]                      # [32, T, 128]
        xTc = xc.transpose(2, 1, 0).reshape(128, n_steps * BL).astype(dt_np)
        wbc = np.ascontiguousarray(np.concatenate([xTc, wtail], axis=1))
        in_maps.append(dict(wb=wbc))
    return in_maps


def run(inputs, dtype="bfloat16", n_steps=T, trace=False, rz8=None):
    if rz8 is None:
        rz8 = os.environ.get("GRU_RZ8", "1") == "1"
    dt_w = F32 if dtype == "float32" else mybir.dt.bfloat16
    dt_np = np.float32 if dtype == "float32" else ml_dtypes.bfloat16
    if dtype == "float32":
        rz8 = False
    key = (dtype, n_steps, rz8)
    if key not in _CACHE:
        nc = _build(dt_w, n_steps, rz8)
        n = _split_multiwaits(nc)
        print(f"split {n} multi-waits", flush=True)
        _CACHE[key] = nc
    nc = _CACHE[key]
    in_maps = _prep_inputs(inputs, dt_np, n_steps, rz8)
    res = run_bass_kernel_spmd(nc, in_maps, list(range(NCORES)), trace=trace)
    outs = [r["out"] for r in res.results]  # each [C, BL]
    full = np.concatenate([o.T for o in outs], axis=0).astype(np.float32)
    return full, res


def kernel(**inputs):
    full, _ = run(inputs, dtype=os.environ.get("GRU_DTYPE", "bfloat16"))
    return full


# revision 8
# speedup vs baseline: 1.3154x; 1.3154x over previous
"""Trainium2 Bass kernel for a 2-layer GRU (B=256, T=256, D=128, H=512) + FC head.

Strategy: data-parallel over batch (B=32 per core, 8 cores), single SPMD launch.
Everything stays on-chip after the initial weight/x loads.

v6: the kernel is recurrence-chain-latency bound, so the design minimizes the
per-step dependency chain:
  - ALL gate accumulation is PSUM-resident in 2-step chunk tiles
    [128, 16(slots), 2(steps), 32]: slots 0:8 r,z / 8:12 xn / 12:16 hn.
    Biases enter PSUM via identity-stationary matmuls (one per bank per
    chunk), the x/y0 projections via chunk GEMMs, and the recurrent matmuls
    accumulate per step on top.
  - Sigmoid runs directly on PSUM (ACT reads PSUM faster than SBUF and the
    DVE pre-add disappears from the chain).
  - h-update: v=1-z and u=z*h are computed right after sigmoid (GpSimd,
    off-chain); after tanh only m1=v*n and h=m1+u remain on the chain.
  - No xg1 SBUF ring / copy: L1 consumes its chunk PSUM directly.

Layouts (per core, local batch 32):
  - gh.T = [gate_dim on partitions, batch free]; h.T as [128, k, 32].
  - x pre-transposed on host: xT [128(D), T, 32(batch)].
  - Weights pre-transposed on host (W.T K-tiles).
"""

import os
import sys

sys.path.insert(0, "/opt/trn_rl_repo")

import ml_dtypes
import numpy as np

import concourse.bass as bass
import concourse.tile as tile
from concourse import mybir
from concourse.bass_utils import run_bass_kernel_spmd

AF = mybir.ActivationFunctionType
ALU = mybir.AluOpType
F32 = mybir.dt.float32

B, T, D, H, C = 256, 256, 128, 512, 10
NCORES = 8
BL = B // NCORES          # 32 batch per core
G3 = 3 * H                # 1536
NK = H // 128             # 4 h k-tiles
NM = G3 // 128            # 12 gate m-tiles
RING = 8                  # y0 ring slots (steps)
CH = 2                    # chunk size (steps)
LAG = 4                   # L1 step lag behind L0

_CACHE = {}


def _split_multiwaits(nc):
    """Walrus/HW allow a single sync-wait per engine instruction. Tile can
    emit several; hoist extras into same-engine NoOps placed just before."""
    import json as _json
    import types as _types

    d = _json.loads(mybir.module_to_json_bytes(nc.m))
    nsw = 0
    for fn in d["functions"]:
        for blk in fn["blocks"]:
            out = []
            for ins in blk["instructions"]:
                si = ins.get("sync_info") or {}
                ow = si.get("on_wait") or []
                if len(ow) > 1:
                    for w in ow[:-1]:
                        out.append({
                            "engine": ins["engine"],
                            "ins": [],
                            "outs": [],
                            "name": f"I-SW-{nsw}",
                            "opcode": "NoOp",
                            "sync_info": {"on_update": [], "on_wait": [w]},
                        })
                        nsw += 1
                    si["on_wait"] = [ow[-1]]
                out.append(ins)
            blk["instructions"] = out
    blob = _json.dumps(d).encode()
    nc.to_json_bytes = _types.MethodType(lambda self: blob, nc)
    return nsw


def _build(dt_w, n_steps):
    """Build the Bass program. dt_w: weight/x/h/gate dtype. Returns nc."""
    DT = dt_w
    nc = bass.Bass("TRN2", target_bir_lowering=False, debug=False, num_devices=NCORES)

    nwf = 2                                # fc1b, fc2b in f32
    fmul = 4 // mybir.dt.size(DT)          # DT cols per f32 value
    nbm = 16 * CH * BL                     # bias moving tile cols per layer
    nwd = (n_steps * BL + G3 + 3 * NK * G3 + NK * 128 + C + 2 + 128
           + 2 * nbm + nwf * fmul)
    d_wb = nc.dram_tensor("wb", [128, nwd], DT, kind="ExternalInput").ap()
    d_out = nc.dram_tensor("out", [C, BL], F32, kind="ExternalOutput").ap()

    with tile.TileContext(nc) as tc:
        with (
            tc.tile_pool(name="w", bufs=1) as wp,
            tc.tile_pool(name="ring", bufs=1) as ringp,
            tc.tile_pool(name="h1", bufs=3) as h1p,
            tc.tile_pool(name="g", bufs=3) as gp,
            tc.tile_pool(name="g2", bufs=3) as gp2,
            tc.tile_pool(name="px0", bufs=2, space="PSUM") as px0p,
            tc.tile_pool(name="px1", bufs=2, space="PSUM") as px1p,
        ):
            # ---- one persistent SBUF blob, ONE load DMA (HW allows only a
            # single sync-wait per instruction, so all consumers join on it) ----
            wb = wp.tile([128, nwd], DT)
            y0r = ringp.tile([128, RING, NK, BL], DT)   # y0 / h0 ring
            h1init = ringp.tile([128, NK, BL], DT)
            nc.sync.dma_start(wb[:], d_wb[:])

            # blob views
            o = 0
            def take(n):
                nonlocal o
                a, o = o, o + n
                return a
            o_xT = take(n_steps * BL)
            o_wih0 = take(G3)
            o_whh0 = take(NK * G3)
            o_wih1 = take(NK * G3)
            o_whh1 = take(NK * G3)
            o_fc1w = take(NK * 128)
            o_fc2w = take(C + 2)
            o_id = take(128)
            o_bm0 = take(nbm)
            o_bm1 = take(nbm)
            o_f32 = take(nwf * fmul)
            xT = wb[:, o_xT:o_xT + n_steps * BL].rearrange("p (t b) -> p t b", b=BL)
            fc2w = wb[:, o_fc2w:o_fc2w + C]
            ident = wb[:, o_id:o_id + 128]
            bm0 = wb[:, o_bm0:o_bm0 + nbm].rearrange(
                "p (m s b) -> p m s b", s=CH, b=BL)
            bm1 = wb[:, o_bm1:o_bm1 + nbm].rearrange(
                "p (m s b) -> p m s b", s=CH, b=BL)
            fbt = wb[:, o_f32:o_f32 + nwf * fmul].bitcast(F32)
            fc1b = fbt[:, 0:1]
            fc2b = fbt[:, 1:2]

            def wih0_t(m):
                return wb[:, o_wih0 + m * 128:o_wih0 + (m + 1) * 128]
            def whh0_t(k, m):
                return wb[:, o_whh0 + k * G3 + m * 128:o_whh0 + k * G3 + (m + 1) * 128]
            def wih1_t(k, m):
                return wb[:, o_wih1 + k * G3 + m * 128:o_wih1 + k * G3 + (m + 1) * 128]
            def whh1_t(k, m):
                return wb[:, o_whh1 + k * G3 + m * 128:o_whh1 + k * G3 + (m + 1) * 128]
            def fc1w_t(k):
                return wb[:, o_fc1w + k * 128:o_fc1w + (k + 1) * 128]

            # HW allows only ONE sync-wait per instruction. Prime DVE/ACT/Pool
            # with a tiny read of the blob so their clocks observe the load-DMA
            # sem once; afterwards every instruction needs at most one wait.
            prdve = gp.tile([1, 4], DT, tag="prime")
            nc.vector.tensor_copy(prdve[:], wb[0:1, 0:4])
            pract = gp.tile([1, 4], DT, tag="prime2")
            nc.scalar.copy(pract[:], wb[0:1, 0:4])
            prgps = gp.tile([1, 4], DT, tag="prime3")
            nc.gpsimd.tensor_copy(prgps[:], wb[0:1, 0:4])
            nc.vector.memset(y0r[:, RING - 1], 0.0)  # h0(t=-1) = 0
            nc.vector.memset(h1init[:], 0.0)

            # m-slot layout in chunk PSUM tiles [128, 16, CH, BL]:
            #   0:8 rz (bias + x/y0-proj + W_hh-rec), 8:12 xn (bias + proj),
            #   12:16 hn (bias + W_hn-rec)
            def l0_chunk(c):
                """bias + x-projection for L0 steps [CH*c, CH*c+CH)."""
                px = px0p.tile([128, 16, CH, BL], F32, tag="px0")
                nc.tensor.matmul(px[:, 0:8], ident, bm0[:, 0:8],
                                 start=True, stop=False)
                nc.tensor.matmul(px[:, 8:16], ident, bm0[:, 8:16],
                                 start=True, stop=False)
                xs = xT[:, CH * c:CH * c + CH]
                for m in range(NM):
                    nc.tensor.matmul(px[:, m], wih0_t(m), xs,
                                     start=False, stop=(8 <= m))
                return px

            def l1_chunk(c):
                """bias + y0-projection for L1 steps [CH*c, CH*c+CH)."""
                r0 = (CH * c) % RING
                px = px1p.tile([128, 16, CH, BL], F32, tag="px1")
                nc.tensor.matmul(px[:, 0:8], ident, bm1[:, 0:8],
                                 start=True, stop=False)
                nc.tensor.matmul(px[:, 8:16], ident, bm1[:, 8:16],
                                 start=True, stop=False)
                for m in range(NM):
                    for k in range(NK):
                        nc.tensor.matmul(
                            px[:, m], wih1_t(k, m), y0r[:, r0:r0 + CH, k],
                            start=False, stop=(8 <= m and k == NK - 1))
                return px

            def gates(px, s, whh_t, h_old, pool, sfx):
                """Shared per-step gate math. h_old: [128, NK, BL] view."""
                for m in range(8):           # r, z rec
                    for k in range(NK):
                        nc.tensor.matmul(
                            px[:, m, s], whh_t(k, m), h_old[:, k],
                            start=False, stop=(k == NK - 1))
                for j in range(NK):          # n rec (hn slots)
                    for k in range(NK):
                        nc.tensor.matmul(
                            px[:, 12 + j, s], whh_t(k, 8 + j), h_old[:, k],
                            start=False, stop=(k == NK - 1))
                sig = pool.tile([128, 8, BL], DT, tag="sig" + sfx)
                nc.scalar.activation(sig[:], px[:, 0:8, s], AF.Sigmoid)
                v = pool.tile([128, NK, BL], DT, tag="v" + sfx)
                nc.gpsimd.tensor_scalar(v[:], sig[:, 4:8], -1.0, 1.0,
                                        ALU.mult, ALU.add)
                u = pool.tile([128, NK, BL], DT, tag="u" + sfx)
                nc.gpsimd.tensor_mul(u[:], sig[:, 4:8], h_old[:])
                t2 = pool.tile([128, NK, BL], DT, tag="t2" + sfx)
                nc.vector.tensor_mul(t2[:], sig[:, 0:4], px[:, 12:16, s])
                t3 = pool.tile([128, NK, BL], DT, tag="t3" + sfx)
                nc.vector.tensor_add(t3[:], t2[:], px[:, 8:12, s])
                ntl = pool.tile([128, NK, BL], DT, tag="ntl" + sfx)
                nc.scalar.activation(ntl[:], t3[:], AF.Tanh)
                m1 = pool.tile([128, NK, BL], DT, tag="m1" + sfx)
                nc.vector.tensor_mul(m1[:], v[:], ntl[:])
                return m1, u

            def l0_step(t, px):
                rs = (t + RING - 1) % RING
                m1, u = gates(px, t % CH, whh0_t, y0r[:, rs], gp, "a")
                nc.vector.tensor_add(y0r[:, t % RING], m1[:], u[:])

            def l1_step(tau, px, h_old):
                m1, u = gates(px, tau % CH, whh1_t, h_old, gp2, "b")
                h_new = h1p.tile([128, NK, BL], DT, tag="h1")
                nc.vector.tensor_add(h_new[:], m1[:], u[:])
                return h_new

            # ---- main pipeline: L1 lags L0 by LAG steps ----
            h1 = h1init
            px0 = px1 = None
            px1q = []
            for t in range(n_steps):
                if t % CH == 0:
                    px0 = l0_chunk(t // CH)
                l0_step(t, px0)
                if t % CH == CH - 1:
                    px1q.append(l1_chunk(t // CH))
                tau = t - LAG
                if tau >= 0:
                    if tau % CH == 0:
                        px1 = px1q.pop(0)
                    h1 = l1_step(tau, px1, h1)
            for tau in range(n_steps - LAG, n_steps):
                if tau % CH == 0:
                    px1 = px1q.pop(0)
                h1 = l1_step(tau, px1, h1)

            # ---- FC head ----
            hr = gp.tile([128, NK, BL], DT, tag="hr")
            nc.scalar.activation(hr[:], h1[:], AF.Relu)
            pfct = px0p.tile([128, 16, CH, BL], F32, tag="px0")
            pfc = pfct[:, 0, 0]
            for k in range(NK):
                nc.tensor.matmul(pfc, fc1w_t(k), hr[:, k],
                                 start=(k == 0), stop=(k == NK - 1))
            o1 = gp.tile([128, BL], DT, tag="o1")
            nc.scalar.activation(o1[:], pfc, AF.Relu, bias=fc1b[:])
            pfc2t = px1p.tile([128, 16, CH, BL], F32, tag="px1")
            pfc2 = pfc2t[0:C, 0, 0]
            nc.tensor.matmul(pfc2, fc2w[:], o1[:], start=True, stop=True)
            ofin = gp.tile([C, BL], F32, tag="ofin")
            nc.scalar.activation(ofin[:], pfc2, AF.Identity, bias=fc2b[0:C, :])
            nc.sync.dma_start(d_out[:], ofin[:])

    return nc


def _prep_inputs(inputs, dt_np, n_steps):
    """Host-side layout prep: pack per-core DT blob."""
    f32 = np.float32
    x = inputs["x"][:, :n_steps, :]
    b_ih0, b_hh0 = inputs["b_ih0"].astype(f32), inputs["b_hh0"].astype(f32)
    b_ih1, b_hh1 = inputs["b_ih1"].astype(f32), inputs["b_hh1"].astype(f32)

    def kt(W):  # [3H, K] -> [128, NKw*3H] K-tiles of W.T side by side
        Wt = W.T.astype(dt_np)  # [K, 3H]
        nk = W.shape[1] // 128
        return Wt.reshape(nk, 128, G3).transpose(1, 0, 2).reshape(128, nk * G3)

    wih0 = inputs["W_ih0"].T.astype(dt_np)               # [128, 1536]
    whh0, wih1, whh1 = kt(inputs["W_hh0"]), kt(inputs["W_ih1"]), kt(inputs["W_hh1"])
    fc1w = (inputs["fc1_w"].T.astype(dt_np)              # [512, 128] -> [128, 4*128]
            .reshape(NK, 128, 128).transpose(1, 0, 2).reshape(128, NK * 128))
    fc2w = inputs["fc2_w"].T.astype(dt_np)               # [128, 10]
    ident = np.eye(128, dtype=dt_np)

    def bm(b_ih, b_hh):
        # [128, 16 m-slots] -> broadcast over (CH, BL) -> [128, 16*CH*BL]
        v = np.zeros((128, 16), f32)
        v[:, 0:8] = (b_ih + b_hh)[:1024].reshape(8, 128).T
        v[:, 8:12] = b_ih[1024:].reshape(4, 128).T
        v[:, 12:16] = b_hh[1024:].reshape(4, 128).T
        return np.ascontiguousarray(np.broadcast_to(
            v[:, :, None, None], (128, 16, CH, BL)
        ).reshape(128, 16 * CH * BL)).astype(dt_np)

    bm0, bm1 = bm(b_ih0, b_hh0), bm(b_ih1, b_hh1)

    fc1b = inputs["fc1_b"].astype(f32).reshape(128, 1)
    fc2b = np.zeros((128, 1), f32)
    fc2b[:C, 0] = inputs["fc2_b"].astype(f32)
    fb = np.ascontiguousarray(np.concatenate([fc1b, fc2b], axis=1)).astype(f32)
    fb_dt = fb.view(np.uint8).reshape(128, -1).view(dt_np)
    pad = np.zeros((128, 2), dt_np)

    wtail = np.concatenate(
        [wih0, whh0, wih1, whh1, fc1w, fc2w, pad, ident, bm0, bm1, fb_dt],
        axis=1)
    in_maps = []
    for c in range(NCORES):
        xc = x[c * BL:(c + 1) * BL]                      # [32, T, 128]
        xTc = xc.transpose(2, 1, 0).reshape(128, n_steps * BL).astype(dt_np)
        wbc = np.ascontiguousarray(np.concatenate([xTc, wtail], axis=1))
        in_maps.append(dict(wb=wbc))
    return in_maps


def run(inputs, dtype="bfloat16", n_steps=T, trace=False):
    dt_w = F32 if dtype == "float32" else mybir.dt.bfloat16
    dt_np = np.float32 if dtype == "float32" else ml_dtypes.bfloat16
    key = (dtype, n_steps)
    if key not in _CACHE:
        nc = _build(dt_w, n_steps)
        n = _split_multiwaits(nc)
        print(f"split {n} multi-waits", flush=True)
        _CACHE[key] = nc
    nc = _CACHE[key]
    in_maps = _prep_inputs(inputs, dt_np, n_steps)
    res = run_bass_kernel_spmd(nc, in_maps, list(range(NCORES)), trace=trace)
    outs = [r["out"] for r in res.results]  # each [C, BL]
    full = np.concatenate([o.T for o in outs], axis=0).astype(np.float32)
    return full, res


def kernel(**inputs):
    full, _ = run(inputs, dtype=os.environ.get("GRU_DTYPE", "bfloat16"))
    return full
